# revision 1
# baseline (speedup 1.0000x reference)
"""Trainium2 Bass kernel for ClassicalSelfAttention.

  out = softmax((X @ R) @ (X @ E).T / sqrt(D)) @ X,  X: (8192, 1024) fp32

Sharding: sequence-parallel over 8 NeuronCores. Core i owns queries
[i*1024, (i+1)*1024). Each core computes its own K^T block (E.T @ X_i^T),
AllGathers the blocks so every core holds full K^T, then runs blocked
flash-style attention over key blocks of 1024 with online softmax merge.

All big matmuls run in float32r (~13-bit mantissa at full PE rate);
accumulation is fp32 in PSUM.
"""
import numpy as np

import concourse.bass as bass_mod
import concourse.bacc as bacc
import concourse.mybir as mybir
from concourse import tile
from concourse.bass_utils import run_bass_kernel_spmd
from concourse.masks import make_identity

DT = mybir.dt
F32 = DT.float32
F32R = DT.float32r
ALU = mybir.AluOpType
ACTF = mybir.ActivationFunctionType

S, D, NCORES = 8192, 1024, 8
SL = S // NCORES          # 1024 queries per core
P = 128                   # partitions
DC = D // P               # 8 contraction chunks
MC = SL // P              # 8 query chunks per core
TB = 1024                 # key block size
NB = S // TB              # 8 key blocks
SCALE = 1.0 / 32.0        # 1/sqrt(D)
NEG_BIG = -1.0e30


def build_program(n_iter=1, bench=None, num_devices=NCORES):
    nc = bacc.Bacc("TRN2", target_bir_lowering=False, debug=False,
                   num_devices=num_devices)

    xt = nc.declare_dram_parameter("xt", [D, SL], F32R, isOutput=False)
    r_p = nc.declare_dram_parameter("r", [D, D], F32R, isOutput=False)
    e_p = nc.declare_dram_parameter("e", [D, D], F32R, isOutput=False)
    x_p = nc.declare_dram_parameter("x", [S, D], F32R, isOutput=False)
    out_p = nc.declare_dram_parameter("out", [SL, D], F32, isOutput=True)

    if bench is None:
        bench = n_iter > 1
    import contextlib
    with tile.TileContext(nc) as tc:
        with (
            tc.tile_pool(name="persist", bufs=1) as pers,
            tc.tile_pool(name="dram", bufs=1, space="DRAM") as dram,
            contextlib.ExitStack() as stack,
        ):
            ktb_own = dram.tile([D, TB], F32R, name="ktb_own")
            ktb_all = dram.tile([NCORES * D, TB], F32R,
                                addr_space="Local" if bench else "Shared",
                                name="ktb_all")
            if bench:
                # touch ktb_all once so in-loop reads see written memory
                nc.sync.dma_start(ktb_all[:], x_p[:].bitcast(F32R))
            if n_iter > 1:
                stack.enter_context(tc.For_i(0, n_iter, 1))

            qt = pers.tile([P, DC * SL], F32R, tag="qt")       # Q^T, [d|m]
            oacc = pers.tile([P, MC * D], F32, tag="oacc")    # O accum per m
            ident32 = pers.tile([P, P], F32, tag="ident32")
            ident = pers.tile([P, P], F32R, tag="ident")
            mst = [[pers.tile([P, 1], F32, tag=f"mst{m}_{j}", name=f"mst{m}_{j}")
                    for j in range(2)] for m in range(MC)]
            sig = [pers.tile([P, 1], F32, tag=f"sig{m}", name=f"sig{m}")
                   for m in range(MC)]

            make_identity(nc, ident32[:])
            nc.vector.tensor_copy(ident[:], ident32[:])
            nc.gpsimd.memset(oacc[:], 0.0)
            for m in range(MC):
                nc.gpsimd.memset(mst[m][0][:], NEG_BIG)
                nc.gpsimd.memset(sig[m][:], 0.0)

            # ---------------- Phase A: projections + allgather ----------
            with (
                tc.tile_pool(name="pa", bufs=1) as pa,
                tc.tile_pool(name="pa_st", bufs=2) as pa_st,
                tc.tile_pool(name="pa_ps", bufs=2, space="PSUM") as pa_ps,
            ):
                xt_sb = pa.tile([P, DC * SL], F32R, tag="xt")   # [d_in | m]
                e_sb = pa.tile([P, DC * D], F32R, tag="re")     # [d_in | d_out]
                kst = pa.tile([P, DC * SL], F32R, tag="kst")    # K^T staging
                nc.sync.dma_start(
                    xt_sb.rearrange("p (k c) -> p k c", k=DC),
                    xt.rearrange("(k p) c -> p k c", p=P))
                nc.sync.dma_start(
                    e_sb.rearrange("p (k c) -> p k c", k=DC),
                    e_p.rearrange("(k p) c -> p k c", p=P))

                # K^T own block: kt_o = E.T @ X_i^T   [d_out, t_local]
                for o in range(DC):
                    ps = pa_ps.tile([P, SL], F32, tag="proj")
                    for k in range(DC):
                        lhsT = e_sb[:, k * D + o * P: k * D + (o + 1) * P]
                        for h in range(SL // 512):
                            nc.tensor.matmul(
                                ps[:, h * 512:(h + 1) * 512],
                                lhsT,
                                xt_sb[:, k * SL + h * 512:
                                      k * SL + (h + 1) * 512],
                                start=(k == 0), stop=(k == DC - 1),
                            )
                    nc.vector.tensor_copy(kst[:, o * SL:(o + 1) * SL], ps[:])

                nc.sync.dma_start(
                    ktb_own.rearrange("(o p) c -> p o c", p=P),
                    kst.rearrange("p (o c) -> p o c", o=DC))

                # R loads into E's slot once the E-projection is done with it
                r_sb = pa.tile([P, DC * D], F32R, tag="re", name="r_sb")
                nc.sync.dma_start(
                    r_sb.rearrange("p (k c) -> p k c", k=DC),
                    r_p.rearrange("(k p) c -> p k c", p=P))

                if bench:
                    # stand-in for the collective with similar local traffic
                    nc.gpsimd.dma_start(ktb_all[0:D, :], ktb_own[:])
                else:
                    nc.gpsimd.collective_compute(
                        "AllGather",
                        ALU.bypass,
                        replica_groups=[list(range(NCORES))],
                        ins=[ktb_own.opt()],
                        outs=[ktb_all.opt()],
                    )

                # Q^T: qt = R.T @ X_i^T   [d_out, m]
                for o in range(DC):
                    ps = pa_ps.tile([P, SL], F32, tag="proj")
                    for k in range(DC):
                        lhsT = r_sb[:, k * D + o * P: k * D + (o + 1) * P]
                        for h in range(SL // 512):
                            nc.tensor.matmul(
                                ps[:, h * 512:(h + 1) * 512],
                                lhsT,
                                xt_sb[:, k * SL + h * 512:
                                      k * SL + (h + 1) * 512],
                                start=(k == 0), stop=(k == DC - 1),
                            )
                    nc.vector.tensor_copy(qt[:, o * SL:(o + 1) * SL], ps[:])

            # ---------------- Phase B: blocked attention -----------------
            # Software-pipelined by one m-step: PE runs transposes+PV of the
            # previous (b, m) while DVE/ACT compute stats+exp of the current.
            with (
                tc.tile_pool(name="kt", bufs=2) as ktp,
                tc.tile_pool(name="xb", bufs=1) as xbp,
                tc.tile_pool(name="ph", bufs=4) as php,
                tc.tile_pool(name="pt", bufs=2) as ptp,
                tc.tile_pool(name="stats", bufs=6) as stp,
                tc.tile_pool(name="s_ps", bufs=4, space="PSUM") as sps,
                tc.tile_pool(name="t_ps", bufs=1, space="PSUM") as tps,
                tc.tile_pool(name="o_ps", bufs=1, space="PSUM") as ops,
            ):
                def flush_pe(pend):
                    ph, alpha, m, b, xb = pend
                    o_part = ops.tile([P, D], F32, tag="opart", name="o_part")
                    tp = tps.tile([P, TB], F32, tag="tp", name="tp")
                    for cc in range(8):
                        nc.tensor.transpose(
                            tp[:, cc * P:(cc + 1) * P].bitcast(F32R),
                            ph[:, cc * P:(cc + 1) * P],
                            ident[:],
                        )
                    pt = ptp.tile([P, TB], F32R, tag="pt", name="pt")
                    nc.scalar.copy(pt[:], tp[:])
                    for cc in range(8):
                        for h in range(D // 512):
                            nc.tensor.matmul(
                                o_part[:, h * 512:(h + 1) * 512],
                                pt[:, cc * P:(cc + 1) * P],
                                xb[:, cc * D + h * 512:
                                   cc * D + (h + 1) * 512],
                                start=(cc == 0), stop=(cc == 7),
                            )
                    return o_part

                def flush_dve(pend, o_part):
                    ph, alpha, m, b, xb = pend
                    nc.vector.scalar_tensor_tensor(
                        oacc[:, m * D:(m + 1) * D],
                        oacc[:, m * D:(m + 1) * D],
                        alpha[:], o_part[:],
                        op0=ALU.mult, op1=ALU.add)
                    if b == NB - 1:
                        # finalize this m: divide by softmax sum and store
                        rcp = stp.tile([P, 1], F32, tag="rcp", name="rcp")
                        nc.vector.reciprocal(rcp[:], sig[m][:])
                        of = php.tile([P, D], F32, tag="ofin", name="ofin")
                        nc.vector.tensor_scalar_mul(
                            of[:], oacc[:, m * D:(m + 1) * D], rcp[:])
                        nc.sync.dma_start(out_p[m * P:(m + 1) * P, :], of[:])

                pending = []
                xb_q = []
                pid = nc.sync.partition_id()
                for b in range(NB):
                    # ring order: process global block (pid + b) % NB; b == 0
                    # is the locally-computed block (no collective dependency)
                    kt = ktp.tile([P, DC * TB], F32R, tag="kt", name="kt")
                    if b == 0:
                        kt_src = ktb_own[:, :]
                    else:
                        kt_src = ktb_all[bass_mod.ds(((pid + b) % NB) * D, D), :]
                    nc.sync.dma_start(
                        kt.rearrange("p (k c) -> p k c", k=DC),
                        kt_src.rearrange("(k p) c -> p k c", p=P))
                    xb = xbp.tile([P, (TB // P) * D], F32R, tag="xb",
                                  name="xb")
                    nc.sync.dma_start(
                        xb.rearrange("p (k c) -> p k c", k=TB // P),
                        x_p[bass_mod.ds(((pid + b) % NB) * TB, TB), :]
                        .rearrange("(k p) c -> p k c", p=P))

                    for m in range(MC):
                        # scores in two 512-halves (h-outer) so stats/exp of
                        # half 0 overlap the matmuls of half 1
                        sh_ = [sps.tile([P, 512], F32, tag="s", name="s")
                               for _ in range(2)]
                        mqh = [stp.tile([P, 1], F32, tag=f"mq{h}",
                                        name=f"mq{h}") for h in range(2)]
                        for h in range(2):
                            for k in range(DC):
                                lhsT = qt[:, k * SL + m * P:
                                          k * SL + (m + 1) * P]
                                nc.tensor.matmul(
                                    sh_[h][:],
                                    lhsT,
                                    kt[:, k * TB + h * 512:
                                       k * TB + (h + 1) * 512],
                                    start=(k == 0), stop=(k == DC - 1),
                                )
                            nc.vector.reduce_max(mqh[h][:], sh_[h][:],
                                                 axis=mybir.AxisListType.X)

                        # online softmax stats; mst ping-pongs on b parity
                        m_old = mst[m][b % 2]
                        mnew = mst[m][(b + 1) % 2]
                        mq = stp.tile([P, 1], F32, tag="mq", name="mq")
                        nc.vector.tensor_max(mq[:], mqh[0][:], mqh[1][:])
                        nc.vector.tensor_max(mnew[:], m_old[:], mq[:])
                        nbias = stp.tile([P, 1], F32, tag="nbias", name="nbias")
                        nc.scalar.mul(nbias[:], mnew[:], -SCALE)
                        # alpha = exp(s*m_old + nbias) = exp((m_old - mnew)/32)
                        alpha = stp.tile([P, 1], F32, tag="alpha", name="alpha")
                        nc.scalar.activation(alpha[:], m_old[:], ACTF.Exp,
                                             bias=nbias[:], scale=SCALE)

                        # phat = exp(s/32 - mnew/32), per half; sums into sq
                        ph = php.tile([P, TB], F32R, tag="ph", name="ph")
                        sqh = [stp.tile([P, 1], F32, tag=f"sq{h}",
                                        name=f"sq{h}") for h in range(2)]
                        for h in range(2):
                            nc.scalar.activation(ph[:, h * 512:(h + 1) * 512],
                                                 sh_[h][:], ACTF.Exp,
                                                 bias=nbias[:], scale=SCALE,
                                                 accum_out=sqh[h][:])
                        sq = stp.tile([P, 1], F32, tag="sq", name="sq")
                        nc.vector.tensor_add(sq[:], sqh[0][:], sqh[1][:])
                        nc.vector.scalar_tensor_tensor(
                            sig[m][:], sig[m][:], alpha[:], sq[:],
                            op0=ALU.mult, op1=ALU.add)

                        pending.append((ph, alpha, m, b, xb))
                        if len(pending) > 2:
                            pend_fl = pending.pop(0)
                            flush_dve(pend_fl, flush_pe(pend_fl))
                for pend in pending:
                    flush_dve(pend, flush_pe(pend))


    nc.compile()
    return nc


_PROGRAM = None


def _get_program():
    global _PROGRAM
    if _PROGRAM is None:
        _PROGRAM = build_program()
    return _PROGRAM


def kernel(inputs, rotation_params, entangle_params, _trace=False):
    X = np.ascontiguousarray(np.asarray(inputs, dtype=np.float32))
    R = np.ascontiguousarray(np.asarray(rotation_params, dtype=np.float32))
    E = np.ascontiguousarray(np.asarray(entangle_params, dtype=np.float32))
    assert X.shape == (S, D) and R.shape == (D, D) and E.shape == (D, D)

    XT = np.ascontiguousarray(X.T)
    in_maps = []
    for i in range(NCORES):
        in_maps.append({
            "xt": np.ascontiguousarray(XT[:, i * SL:(i + 1) * SL]),
            "r": R,
            "e": E,
            "x": X,
        })

    nc = _get_program()
    res = run_bass_kernel_spmd(nc, in_maps, list(range(NCORES)),
                               trace=_trace)
    out = np.concatenate([res.results[i]["out"] for i in range(NCORES)],
                         axis=0)
    if _trace:
        return out, res
    return out



# revision 5
# speedup vs baseline: 1.0495x; 1.0495x over previous
"""Trainium2 Bass kernel for ClassicalSelfAttention.

  out = softmax((X @ R) @ (X @ E).T / sqrt(D)) @ X,  X: (8192, 1024) fp32

Sharding: sequence-parallel over 8 NeuronCores. Core i owns queries
[i*1024, (i+1)*1024).

Uses scores = X (R E^T) X^T: each core AllGathers the raw X^T blocks
(collective starts at t=0, no compute dependency), and meanwhile computes
H = R^T X_i^T then G^T = E^T^T H = (X_i R E^T)^T locally. Scores for key
block b contract G^T against the gathered X^T block directly, so no
per-block key projection is needed. Block 0 (own block) reads X_i^T from
the input param, fully hiding the collective.

Score matmuls run in float32r (~13-bit mantissa, full PE rate) — needed
because softmax gaps are O(1) while scores are O(1000). The probability
matrix P and the PV matmul run in bf16 (P in [0,1], X replicated in bf16
by the host), which halves weight-load time and PV DMA traffic.
"""
import numpy as np
import ml_dtypes

import concourse.bass as bass_mod
import concourse.bacc as bacc
import concourse.mybir as mybir
from concourse import tile
from concourse.bass_utils import run_bass_kernel_spmd
from concourse.masks import make_identity

DT = mybir.dt
F32 = DT.float32
F32R = DT.float32r
BF16 = DT.bfloat16
ALU = mybir.AluOpType
ACTF = mybir.ActivationFunctionType

S, D, NCORES = 8192, 1024, 8
SL = S // NCORES          # 1024 queries per core
P = 128                   # partitions
DC = D // P               # 8 contraction chunks
MC = SL // P              # 8 query chunks per core
TB = 1024                 # key block size
NB = S // TB              # 8 key blocks
SCALE = 1.0 / 32.0        # 1/sqrt(D)
NEG_BIG = -1.0e30


def build_program(num_devices=NCORES):
    nc = bacc.Bacc("TRN2", target_bir_lowering=False, debug=False,
                   num_devices=num_devices)

    xt = nc.declare_dram_parameter("xt", [D, SL], F32R, isOutput=False)
    r_p = nc.declare_dram_parameter("r", [D, D], F32R, isOutput=False)
    et_p = nc.declare_dram_parameter("et", [D, D], F32R, isOutput=False)
    xb_p = nc.declare_dram_parameter("xb16", [S, D], BF16, isOutput=False)
    out_p = nc.declare_dram_parameter("out", [SL, D], F32, isOutput=True)

    with tile.TileContext(nc) as tc:
        with (
            tc.tile_pool(name="persist", bufs=1) as pers,
            tc.tile_pool(name="dram", bufs=1, space="DRAM") as dram,
        ):
            xtall = dram.tile([NCORES * D, SL], F32R, addr_space="Shared",
                              name="xtall")
            xt_own = dram.tile([D, SL], F32R, name="xt_own")

            g = pers.tile([P, DC * SL], F32R, tag="g")        # G^T, [w | q]
            oacc = pers.tile([P, MC * D], F32, tag="oacc")    # O accum per m
            ident32 = pers.tile([P, P], F32, tag="ident32")
            ident = pers.tile([P, P], BF16, tag="ident")
            mst = [[pers.tile([P, 1], F32, tag=f"mst{m}_{j}", name=f"mst{m}_{j}")
                    for j in range(2)] for m in range(MC)]
            sig = [pers.tile([P, 1], F32, tag=f"sig{m}", name=f"sig{m}")
                   for m in range(MC)]

            # ------- collective first: only a DRAM->DRAM stage before it --
            nc.sync.dma_start(xt_own[:], xt[:])
            nc.gpsimd.collective_compute(
                "AllGather",
                ALU.bypass,
                replica_groups=[list(range(NCORES))],
                ins=[xt_own.opt()],
                outs=[xtall.opt()],
            )

            make_identity(nc, ident32[:])
            nc.vector.tensor_copy(ident[:], ident32[:])
            nc.gpsimd.memset(oacc[:], 0.0)
            for m in range(MC):
                nc.gpsimd.memset(mst[m][0][:], NEG_BIG)
                nc.gpsimd.memset(sig[m][:], 0.0)

            # ---------------- Phase A: G^T = (X_i R E^T)^T ---------------
            with (
                tc.tile_pool(name="pa", bufs=1) as pa,
                tc.tile_pool(name="pa_ps", bufs=2, space="PSUM") as pa_ps,
            ):
                r_sb = pa.tile([P, DC * D], F32R, tag="r")     # R  [d | c]
                et_sb = pa.tile([P, DC * D], F32R, tag="et")   # E^T [c | w]
                xt_sb = pa.tile([P, DC * SL], F32R, tag="xt")  # X_i^T [d | q]
                h_sb = pa.tile([P, DC * SL], F32R, tag="h")    # H [c | q]
                # per-chunk DMAs so compute starts as soon as chunk 0 lands
                for k in range(DC):
                    nc.sync.dma_start(
                        r_sb[:, k * D:(k + 1) * D],
                        r_p[k * P:(k + 1) * P, :])
                    nc.sync.dma_start(
                        xt_sb[:, k * SL:(k + 1) * SL],
                        xt[k * P:(k + 1) * P, :])
                    nc.sync.dma_start(
                        et_sb[:, k * D:(k + 1) * D],
                        et_p[k * P:(k + 1) * P, :])

                # H = R^T @ X_i^T  [c, q]
                for o in range(DC):
                    ps = pa_ps.tile([P, SL], F32, tag="proj")
                    for h in range(SL // 512):
                        for k in range(DC):
                            nc.tensor.matmul(
                                ps[:, h * 512:(h + 1) * 512],
                                r_sb[:, k * D + o * P: k * D + (o + 1) * P],
                                xt_sb[:, k * SL + h * 512:
                                      k * SL + (h + 1) * 512],
                                start=(k == 0), stop=(k == DC - 1),
                            )
                    nc.vector.tensor_copy(h_sb[:, o * SL:(o + 1) * SL], ps[:])

                # G^T = E H  [w, q]   (lhsT = E^T chunks)
                for o in range(DC):
                    ps = pa_ps.tile([P, SL], F32, tag="proj")
                    for h in range(SL // 512):
                        for k in range(DC):
                            nc.tensor.matmul(
                                ps[:, h * 512:(h + 1) * 512],
                                et_sb[:, k * D + o * P: k * D + (o + 1) * P],
                                h_sb[:, k * SL + h * 512:
                                     k * SL + (h + 1) * 512],
                                start=(k == 0), stop=(k == DC - 1),
                            )
                    nc.vector.tensor_copy(g[:, o * SL:(o + 1) * SL], ps[:])

            # ---------------- Phase B: blocked attention -----------------
            # Software-pipelined by one m-step: PE runs transposes+PV of the
            # previous (b, m) while DVE/ACT compute stats+exp of the current.
            with (
                tc.tile_pool(name="kt", bufs=2) as ktp,
                tc.tile_pool(name="xb", bufs=2) as xbp,
                tc.tile_pool(name="ph", bufs=4) as php,
                tc.tile_pool(name="pt", bufs=2) as ptp,
                tc.tile_pool(name="ofin", bufs=2) as ofp,
                tc.tile_pool(name="stats", bufs=6) as stp,
                tc.tile_pool(name="s_ps", bufs=4, space="PSUM") as sps,
                tc.tile_pool(name="t_ps", bufs=2, space="PSUM") as tps,
                tc.tile_pool(name="o_ps", bufs=1, space="PSUM") as ops,
            ):
                def flush_pe(pend):
                    ph, alpha, m, b, xb = pend
                    o_part = ops.tile([P, D], F32, tag="opart", name="o_part")
                    tp = tps.tile([P, TB], BF16, tag="tp", name="tp")
                    for cc in range(8):
                        nc.tensor.transpose(
                            tp[:, cc * P:(cc + 1) * P],
                            ph[:, cc * P:(cc + 1) * P],
                            ident[:],
                        )
                    pt = ptp.tile([P, TB], BF16, tag="pt", name="pt")
                    nc.scalar.copy(pt[:], tp[:])
                    for cc in range(8):
                        for h in range(D // 512):
                            nc.tensor.matmul(
                                o_part[:, h * 512:(h + 1) * 512],
                                pt[:, cc * P:(cc + 1) * P],
                                xb[:, cc * D + h * 512:
                                   cc * D + (h + 1) * 512],
                                start=(cc == 0), stop=(cc == 7),
                            )
                    return o_part

                def flush_dve(pend, o_part):
                    ph, alpha, m, b, xb = pend
                    nc.vector.scalar_tensor_tensor(
                        oacc[:, m * D:(m + 1) * D],
                        oacc[:, m * D:(m + 1) * D],
                        alpha[:], o_part[:],
                        op0=ALU.mult, op1=ALU.add)
                    if b == NB - 1:
                        # finalize this m: divide by softmax sum and store
                        rcp = stp.tile([P, 1], F32, tag="rcp", name="rcp")
                        nc.vector.reciprocal(rcp[:], sig[m][:])
                        of = ofp.tile([P, D], F32, tag="ofin", name="ofin")
                        nc.vector.tensor_scalar_mul(
                            of[:], oacc[:, m * D:(m + 1) * D], rcp[:])
                        nc.sync.dma_start(out_p[m * P:(m + 1) * P, :], of[:])

                pending = []
                pid = nc.sync.partition_id()
                for b in range(NB):
                    # ring order: process global block (pid + b) % NB; b == 0
                    # is the local block (reads the xt param, no collective
                    # dependency)
                    kt = ktp.tile([P, DC * TB], F32R, tag="kt", name="kt")
                    if b == 0:
                        for k in range(DC):
                            nc.sync.dma_start(
                                kt[:, k * TB:(k + 1) * TB],
                                xt[k * P:(k + 1) * P, :])
                    else:
                        kt_src = xtall[bass_mod.ds(((pid + b) % NB) * D, D), :]
                        nc.sync.dma_start(
                            kt.rearrange("p (k c) -> p k c", k=DC),
                            kt_src.rearrange("(k p) c -> p k c", p=P))
                    xb = xbp.tile([P, (TB // P) * D], BF16, tag="xb",
                                  name="xb")
                    nc.sync.dma_start(
                        xb.rearrange("p (k c) -> p k c", k=TB // P),
                        xb_p[bass_mod.ds(((pid + b) % NB) * TB, TB), :]
                        .rearrange("(k p) c -> p k c", p=P))

                    for m in range(MC):
                        # scores in two 512-halves (h-outer) so stats/exp of
                        # half 0 overlap the matmuls of half 1
                        sh_ = [sps.tile([P, 512], F32, tag="s", name="s")
                               for _ in range(2)]
                        mqh = [stp.tile([P, 1], F32, tag=f"mq{h}",
                                        name=f"mq{h}") for h in range(2)]
                        for h in range(2):
                            for k in range(DC):
                                lhsT = g[:, k * SL + m * P:
                                         k * SL + (m + 1) * P]
                                nc.tensor.matmul(
                                    sh_[h][:],
                                    lhsT,
                                    kt[:, k * TB + h * 512:
                                       k * TB + (h + 1) * 512],
                                    start=(k == 0), stop=(k == DC - 1),
                                )
                            nc.vector.reduce_max(mqh[h][:], sh_[h][:],
                                                 axis=mybir.AxisListType.X)

                        # online softmax stats; mst ping-pongs on b parity
                        m_old = mst[m][b % 2]
                        mnew = mst[m][(b + 1) % 2]
                        mq = stp.tile([P, 1], F32, tag="mq", name="mq")
                        nc.vector.tensor_max(mq[:], mqh[0][:], mqh[1][:])
                        nc.vector.tensor_max(mnew[:], m_old[:], mq[:])
                        nbias = stp.tile([P, 1], F32, tag="nbias", name="nbias")
                        nc.scalar.mul(nbias[:], mnew[:], -SCALE)
                        # alpha = exp(s*m_old + nbias) = exp((m_old - mnew)/32)
                        alpha = stp.tile([P, 1], F32, tag="alpha", name="alpha")
                        nc.scalar.activation(alpha[:], m_old[:], ACTF.Exp,
                                             bias=nbias[:], scale=SCALE)

                        # phat = exp(s/32 - mnew/32), per half; sums into sq
                        ph = php.tile([P, TB], BF16, tag="ph", name="ph")
                        sqh = [stp.tile([P, 1], F32, tag=f"sq{h}",
                                        name=f"sq{h}") for h in range(2)]
                        for h in range(2):
                            nc.scalar.activation(ph[:, h * 512:(h + 1) * 512],
                                                 sh_[h][:], ACTF.Exp,
                                                 bias=nbias[:], scale=SCALE,
                                                 accum_out=sqh[h][:])
                        sq = stp.tile([P, 1], F32, tag="sq", name="sq")
                        nc.vector.tensor_add(sq[:], sqh[0][:], sqh[1][:])
                        nc.vector.scalar_tensor_tensor(
                            sig[m][:], sig[m][:], alpha[:], sq[:],
                            op0=ALU.mult, op1=ALU.add)

                        pending.append((ph, alpha, m, b, xb))
                        if len(pending) > 2:
                            pend_fl = pending.pop(0)
                            flush_dve(pend_fl, flush_pe(pend_fl))
                for pend in pending:
                    flush_dve(pend, flush_pe(pend))

    nc.compile()
    return nc


_PROGRAM = None


def _get_program():
    global _PROGRAM
    if _PROGRAM is None:
        _PROGRAM = build_program()
    return _PROGRAM


def kernel(inputs, rotation_params, entangle_params, _trace=False):
    X = np.ascontiguousarray(np.asarray(inputs, dtype=np.float32))
    R = np.ascontiguousarray(np.asarray(rotation_params, dtype=np.float32))
    E = np.ascontiguousarray(np.asarray(entangle_params, dtype=np.float32))
    assert X.shape == (S, D) and R.shape == (D, D) and E.shape == (D, D)

    XT = np.ascontiguousarray(X.T)
    ET = np.ascontiguousarray(E.T)
    X16 = X.astype(ml_dtypes.bfloat16)
    in_maps = []
    for i in range(NCORES):
        in_maps.append({
            "xt": np.ascontiguousarray(XT[:, i * SL:(i + 1) * SL]),
            "r": R,
            "et": ET,
            "xb16": X16,
        })

    nc = _get_program()
    res = run_bass_kernel_spmd(nc, in_maps, list(range(NCORES)),
                               trace=_trace)
    out = np.concatenate([res.results[i]["out"] for i in range(NCORES)],
                         axis=0)
    if _trace:
        return out, res
    return out
